# revision 74
# baseline (speedup 1.0000x reference)
"""MoE routing + expert MLP (DynSIHAMLP) on 8 TRN2 NeuronCores.

Strategy (expert-parallel, two device phases):
  Phase 1 (data-parallel router): each core takes T/8 = 2048 tokens,
    computes logits = x @ Wg in fp32 on the TensorEngine, softmax + top-2
    + renormalized weights on Vector/Scalar engines, and per-expert
    count/prob partial sums for the aux loss.
  Host dispatch: tokens are grouped by expert id (pure data movement),
    padded to a per-expert capacity chosen from the actual max count.
  Phase 2 (expert-parallel FFN): core c owns experts {2c, 2c+1}; computes
    y = silu(x @ W1 + b1) @ W2 + b2, scaled by the combine weight, in bf16
    on the TensorEngine (fp32 PSUM accumulation). Core 0 also computes the
    aux loss from the globally-summed router stats.
  Host unshard: scatter-add the two weighted expert outputs per token.
"""

import time

import numpy as np
import ml_dtypes
from contextlib import ExitStack

import concourse.bass as bass
import concourse.bacc as bacc
import concourse.mybir as mybir
import concourse.tile as tile
from concourse import bass_utils

dt = mybir.dt
AF = mybir.ActivationFunctionType
ALU = mybir.AluOpType
AX = mybir.AxisListType

B, S, H, E, F = 4, 4096, 1024, 16, 4096
T = B * S              # 16384 tokens
TOPK = 2
NCORES = 8
TPC = T // NCORES      # 2048 tokens per core in phase 1
EPC = E // NCORES      # 2 experts per core in phase 2
HK = H // 128          # 8 K-tiles over H
FK = F // 128          # 32 tiles over F
NT = TPC // 128        # 16 token tiles per core in phase 1


def build_phase1(xgrp=2, first_split=False, inc_out=True):
    """Router: per-core logits, softmax, top-2, weights, aux partial sums.

    xgrp: token tiles per x-load DMA (trade startup latency vs DMA count).
    first_split: load the first group's tiles individually so the first
      router matmul starts after a half-size DMA.
    """
    nc = bacc.Bacc("TRN2", target_bir_lowering=False, debug=False,
                   num_devices=NCORES)
    xT = nc.dram_tensor("xT", [H, TPC], dt.float32, kind="ExternalInput").ap()
    wg = nc.dram_tensor("wg", [H, E], dt.float32, kind="ExternalInput").ap()
    ti = nc.dram_tensor("ti", [TPC, TOPK], dt.uint32, kind="ExternalOutput").ap()
    tw = nc.dram_tensor("tw", [TPC, TOPK], dt.float32, kind="ExternalOutput").ap()
    # per-partition partial sums; the host finishes the reduction while
    # unsharding (rows 0..127 = dispatch counts, 128..255 = prob sums)
    stats = nc.dram_tensor("stats", [2 * 128, E], dt.float32,
                           kind="ExternalOutput").ap()

    with tile.TileContext(nc) as tc:
        with ExitStack() as ctx:
            const = ctx.enter_context(tc.tile_pool(name="const", bufs=1))
            work = ctx.enter_context(tc.tile_pool(name="work", bufs=6))
            accp = ctx.enter_context(tc.tile_pool(name="accp", bufs=1))
            psum = ctx.enter_context(tc.tile_pool(name="psum", bufs=4, space="PSUM"))
            psum_s = ctx.enter_context(tc.tile_pool(name="psum_s", bufs=1, space="PSUM"))

            wgs = const.tile([128, HK, E], dt.float32)
            nc.sync.dma_start(wgs[:], wg.rearrange("(hk p) e -> p hk e", p=128))
            xts = const.tile([128, NT // xgrp, HK, 128 * xgrp], dt.float32)
            for g in range(NT // xgrp):
                if g == 0 and first_split:
                    for s in range(xgrp):
                        nc.sync.dma_start(
                            xts[:, 0, :, s * 128:(s + 1) * 128],
                            xT[:, s * 128:(s + 1) * 128]
                            .rearrange("(hk p) t -> p hk t", p=128))
                    continue
                nc.sync.dma_start(
                    xts[:, g],
                    xT[:, g * 128 * xgrp:(g + 1) * 128 * xgrp]
                    .rearrange("(hk p) t -> p hk t", p=128))

            iota_i = const.tile([128, E], dt.int32)
            nc.gpsimd.iota(iota_i[:], pattern=[[1, E]], base=0, channel_multiplier=0)
            iota_f = const.tile([128, E], dt.float32)
            nc.vector.tensor_copy(iota_f[:], iota_i[:])

            cnt_acc = accp.tile([128, E], dt.float32)
            nc.vector.memset(cnt_acc[:], 0.0)
            prob_acc = accp.tile([128, E], dt.float32)
            nc.vector.memset(prob_acc[:], 0.0)
            ti_all = accp.tile([128, NT, TOPK], dt.uint32)
            tw_all = accp.tile([128, NT, TOPK], dt.float32)

            for i in range(NT):
                pl = psum.tile([128, E], dt.float32)
                for hk in range(HK):
                    nc.tensor.matmul(
                        pl[:],
                        xts[:, i // xgrp, hk,
                            (i % xgrp) * 128:(i % xgrp + 1) * 128],
                        wgs[:, hk, :],
                        start=(hk == 0), stop=(hk == HK - 1),
                    )
                negmax = work.tile([128, 1], dt.float32)
                nc.vector.tensor_reduce(negmax[:], pl[:], axis=AX.X,
                                        op=ALU.max, negate=True)
                expt = work.tile([128, E], dt.float32)
                nc.scalar.activation(expt[:], pl[:], AF.Exp,
                                     bias=negmax[:], scale=1.0)
                ssum = work.tile([128, 1], dt.float32)
                nc.vector.reduce_sum(ssum[:], expt[:], axis=AX.X)
                rs = work.tile([128, 1], dt.float32)
                nc.vector.reciprocal(rs[:], ssum[:])
                # prob_acc += expt * rs, fused; top-k runs in the exp domain
                # (same order as probs; the 1/Z factor cancels in tw)
                nc.vector.scalar_tensor_tensor(
                    prob_acc[:], expt[:], rs[:], prob_acc[:],
                    ALU.mult, ALU.add)

                vals8 = work.tile([128, 8], dt.float32)
                nc.vector.max(vals8[:], expt[:])
                idx8 = work.tile([128, 8], dt.uint32)
                nc.vector.max_index(idx8[:], vals8[:], expt[:])
                nc.vector.tensor_copy(ti_all[:, i, :], idx8[:, 0:TOPK])

                s2 = work.tile([128, 1], dt.float32)
                nc.vector.reduce_sum(s2[:], vals8[:, 0:TOPK], axis=AX.X)
                rs2 = work.tile([128, 1], dt.float32)
                nc.vector.reciprocal(rs2[:], s2[:])
                nc.scalar.mul(tw_all[:, i, :], vals8[:, 0:TOPK], rs2[:])

                idxf = work.tile([128, TOPK], dt.float32)
                nc.vector.tensor_copy(idxf[:], idx8[:, 0:TOPK])
                for k in range(TOPK):
                    nc.vector.scalar_tensor_tensor(
                        cnt_acc[:], iota_f[:], idxf[:, k:k + 1], cnt_acc[:],
                        ALU.is_equal, ALU.add)

                if inc_out and i % 4 == 3:
                    s = slice(i - 3, i + 1)
                    nc.sync.dma_start(
                        ti.rearrange("(i p) k -> p i k", p=128)[:, s], ti_all[:, s])
                    nc.sync.dma_start(
                        tw.rearrange("(i p) k -> p i k", p=128)[:, s], tw_all[:, s])

            nc.sync.dma_start(stats[0:128, :], cnt_acc[:])
            nc.sync.dma_start(stats[128:256, :], prob_acc[:])
            if not inc_out:
                nc.sync.dma_start(ti.rearrange("(i p) k -> p i k", p=128), ti_all[:])
                nc.sync.dma_start(tw.rearrange("(i p) k -> p i k", p=128), tw_all[:])
    nc.compile()
    return nc


def build_phase2(caps, use_b1=True, use_b2=True, bufs=(2, 3, 4, 4), repeat=1,
                 nq=16, nmax=None, mrun=None, smf=False):
    """Expert FFN: 2 experts per core, bf16 matmuls, fp32 accumulation.

    caps: per-slot token capacities (EPC ints, multiples of 128, compile-time).
    use_b1/use_b2: emit bias adds (skipped when the biases are all zero).
    nmax: per-slot exact max valid-token counts; mm1/activations stop there
      (only sound when every slot's first chunk is full, i.e. cap >= 512,
      so stale hs columns hold finite bf16 from a prior full chunk).
    mrun: per-slot optional (lo, hi) core-id interval that needs the slot's
      LAST output m-tile; other cores branch over it (their yg rows beyond
      their own token count stay zero and are never read by the host).
    """
    if isinstance(caps, int):
        caps = [caps] * EPC
    if nmax is None or any(n < 512 for n in nmax):
        nmax = list(caps)
    if mrun is None:
        mrun = [None] * EPC
    sf0 = smf
    nc = bacc.Bacc("TRN2", target_bir_lowering=False, debug=False,
                   num_devices=NCORES)
    xgT, gw, yg = [], [], []
    for j in range(EPC):
        xgT.append(nc.dram_tensor(f"xgT{j}", [H, caps[j]], dt.bfloat16,
                                  kind="ExternalInput").ap())
        gw.append(nc.dram_tensor(f"gw{j}", [1, caps[j]], dt.float32,
                                 kind="ExternalInput").ap())
        yg.append(nc.dram_tensor(f"yg{j}", [caps[j], H], dt.float32,
                                 kind="ExternalOutput").ap())
    w1 = nc.dram_tensor("w1", [EPC, H, F], dt.bfloat16, kind="ExternalInput").ap()
    b1 = nc.dram_tensor("b1", [EPC, F], dt.float32, kind="ExternalInput").ap()
    w2 = nc.dram_tensor("w2", [EPC, F, H], dt.bfloat16, kind="ExternalInput").ap()
    b2 = nc.dram_tensor("b2", [EPC, H], dt.bfloat16, kind="ExternalInput").ap()
    sg = nc.dram_tensor("sg", [1, 2 * E], dt.float32, kind="ExternalInput").ap()
    aux = nc.dram_tensor("aux", [1, 1], dt.float32, kind="ExternalOutput").ap()

    def chunk_list(cap, small_first=False):
        chunks, off = [], 0
        if small_first and cap > 512:
            chunks.append((0, 128))
            off = 128
        while off < cap:
            cn = min(512, cap - off)
            chunks.append((off, cn))
            off += cn
        return chunks

    with tile.TileContext(nc) as tc:
        with ExitStack() as ctx:
            xb, yb, p1b, p2b = bufs
            wpool = ctx.enter_context(tc.tile_pool(name="wpool", bufs=1))
            cpool = ctx.enter_context(tc.tile_pool(name="cpool", bufs=1))
            xpool = ctx.enter_context(tc.tile_pool(name="xpool", bufs=xb))
            hpool = ctx.enter_context(tc.tile_pool(name="hpool", bufs=1))
            ypool = ctx.enter_context(tc.tile_pool(name="ypool", bufs=yb))
            spool = ctx.enter_context(tc.tile_pool(name="spool", bufs=2))
            psum1 = ctx.enter_context(tc.tile_pool(name="psum1", bufs=p1b, space="PSUM"))
            psum2 = ctx.enter_context(tc.tile_pool(name="psum2", bufs=p2b, space="PSUM"))

            if any(iv is not None and (iv[0] > 0 or iv[1] < NCORES - 1)
                   for iv in mrun):
                nc.cache_partition_id()

            if use_b2:
                ones_bf = cpool.tile([1, 128], dt.bfloat16)
                nc.vector.memset(ones_bf[:], 1.0)

            # aux loss on every core (host reads core 0's)
            sgt = cpool.tile([1, 2 * E], dt.float32)
            nc.sync.dma_start(sgt[:], sg[:])
            prod = cpool.tile([1, E], dt.float32)
            nc.vector.tensor_tensor(prod[:], sgt[:, 0:E], sgt[:, E:2 * E], ALU.mult)
            psc = cpool.tile([1, 1], dt.float32)
            nc.vector.reduce_sum(psc[:], prod[:], axis=AX.X)
            auxt = cpool.tile([1, 1], dt.float32)
            nc.scalar.mul(auxt[:], psc[:], float(E) / (TOPK * T * T))
            nc.sync.dma_start(aux[:], auxt[:])

            NQ = nq           # W1 split into NQ pieces along F
            FQ = FK // NQ     # f-tiles per W1 piece
            FH = FK // 2      # W2 split into halves along F
            for e in [e for _ in range(repeat) for e in range(EPC)]:
                def w1_quarter(q, e=e):
                    t = wpool.tile([128, HK, F // NQ], dt.bfloat16, tag=f"w1q{q}")
                    nc.sync.dma_start(
                        t[:],
                        w1[e, :, q * (F // NQ):(q + 1) * (F // NQ)]
                        .rearrange("(hk p) f -> p hk f", p=128))
                    return t

                # only the first W1 quarter gates the first matmul; the rest
                # of the weights stream in behind the first token chunk
                w1q = [w1_quarter(0)]
                w2h = []
                first = True
                # hs per expert (later chunks' unwritten tail columns must
                # alias the previous chunk's finite values with tracked
                # dependencies), split in two f-halves so mm2's first half
                # doesn't wait on the last f-tile's activation
                hs_a = hpool.tile([128, FH, 512], dt.bfloat16, tag="hs_a")
                hs_b = hpool.tile([128, FH, 512], dt.bfloat16, tag="hs_b")
                hs2 = [hs_a, hs_b]
                for (c0, cn) in chunk_list(caps[e], small_first=(e == 0 and sf0)):
                    # valid-token count for this chunk (cv <= cn; pads beyond
                    # cv are skipped in mm1, zeroed by gw=0 in mm2's output)
                    cv = max(1, min(cn, nmax[e] - c0))
                    xt = xpool.tile([128, HK, 512], dt.bfloat16, tag="xt")
                    nc.sync.dma_start(
                        xt[:, :, 0:cv],
                        xgT[e][:, c0:c0 + cv].rearrange("(hk p) t -> p hk t", p=128))
                    gwt = spool.tile([128, 4], dt.float32, tag="gwt")
                    nm = cn // 128
                    nc.sync.dma_start(
                        gwt[:, 0:nm],
                        gw[e][0, c0:c0 + cn].rearrange("(s p) -> p s", p=128))

                    if first:
                        first = False
                        for q in range(1, NQ):
                            w1q.append(w1_quarter(q))
                        for half in range(2):
                            w2t = wpool.tile([128, FH, H], dt.bfloat16,
                                             tag=f"w2t{half}")
                            nc.sync.dma_start(
                                w2t[:],
                                w2[e, half * (F // 2):(half + 1) * (F // 2), :]
                                .rearrange("(fk p) h -> p fk h", p=128))
                            w2h.append(w2t)
                        if use_b1:
                            b1t = spool.tile([128, FK], dt.float32, tag="b1t")
                            nc.sync.dma_start(
                                b1t[:], b1[e].rearrange("(fk p) -> p fk", p=128))
                        if use_b2:
                            b2t = spool.tile([1, H], dt.bfloat16, tag="b2t")
                            nc.sync.dma_start(b2t[:], b2[e:e + 1, :])

                    for ft in range(FK):
                        w1s = w1q[ft // FQ][:, :, (ft % FQ) * 128:(ft % FQ + 1) * 128]
                        ph = psum1.tile([128, 512], dt.float32, tag="ph")
                        for hk in range(HK):
                            nc.tensor.matmul(
                                ph[:, 0:cv],
                                w1s[:, hk, :],
                                xt[:, hk, 0:cv],
                                start=(hk == 0), stop=(hk == HK - 1))
                        # silu(z) = z * sigmoid(z), z = ph + b1
                        b1ap = b1t[:, ft:ft + 1] if use_b1 else 0.0
                        sig = ypool.tile([128, 512], dt.float32, tag="sig")
                        nc.scalar.activation(sig[:, 0:cv], ph[:, 0:cv],
                                             AF.Sigmoid, bias=b1ap,
                                             scale=1.0)
                        nc.vector.scalar_tensor_tensor(
                            hs2[ft // FH][:, ft % FH, 0:cv], ph[:, 0:cv], b1ap,
                            sig[:, 0:cv], ALU.add, ALU.mult)

                    for m in range(nm):
                        def mtile(m=m, c0=c0, e=e, gwt=gwt, hs2=hs2, w2h=w2h,
                                  b2t=(b2t if use_b2 else None)):
                            for nh in range(2):
                                py = psum2.tile([128, 512], dt.float32, tag="py")
                                for fk in range(FK):
                                    nc.tensor.matmul(
                                        py[:],
                                        hs2[fk // FH][:, fk % FH,
                                                      m * 128:(m + 1) * 128],
                                        w2h[fk // FH][:, fk % FH,
                                                      nh * 512:(nh + 1) * 512],
                                        start=(fk == 0),
                                        stop=(not use_b2 and fk == FK - 1))
                                if use_b2:
                                    nc.tensor.matmul(
                                        py[:], ones_bf[:],
                                        b2t[:, nh * 512:(nh + 1) * 512],
                                        start=False, stop=True)
                                ysb = ypool.tile([128, 512], dt.float32, tag="ysb")
                                nc.scalar.mul(ysb[:], py[:], gwt[:, m:m + 1])
                                nc.sync.dma_start(
                                    yg[e][c0 + m * 128:c0 + (m + 1) * 128,
                                          nh * 512:(nh + 1) * 512],
                                    ysb[:])

                        is_last_mtile = (c0 + (m + 1) * 128 == caps[e])
                        iv = mrun[e]
                        if (is_last_mtile and iv is not None
                                and (iv[0] > 0 or iv[1] < NCORES - 1)):
                            pid = nc.partition_id()
                            cond = (pid < iv[1] + 1 if iv[0] == 0
                                    else pid > iv[0] - 1)
                            with tc.If(cond):
                                mtile()
                        else:
                            mtile()
    nc.compile()
    return nc


LAST_EXEC_NS = []
LAST_TRACES = []
LAST_TI = None
_BUILD_CACHE = {}


def _run(nc, in_maps, **kw):
    last_exc = None
    for attempt in range(3):
        if attempt:
            time.sleep(30 * attempt)
        try:
            r = bass_utils.run_bass_kernel_spmd(
                nc, in_maps, core_ids=list(range(NCORES)), **kw)
            break
        except Exception as exc:  # transient device wedges (NRT_* unrecoverable)
            last_exc = exc
    else:
        raise last_exc
    if r.exec_time_ns is not None:
        LAST_EXEC_NS.append(r.exec_time_ns)
    if r.instructions_and_trace is not None:
        LAST_TRACES.append(r.instructions_and_trace[1])
    return r


def kernel(x, Wg, W1, b1, W2, b2, _trace=False, _prebuilt=None):
    LAST_EXEC_NS.clear()
    LAST_TRACES.clear()
    x = np.asarray(x, dtype=np.float32)
    Wg = np.asarray(Wg, dtype=np.float32)
    W1 = np.asarray(W1, dtype=np.float32)
    b1 = np.asarray(b1, dtype=np.float32)
    W2 = np.asarray(W2, dtype=np.float32)
    b2 = np.asarray(b2, dtype=np.float32)
    xt = x.reshape(T, H)

    # ---- phase 1: router ----
    if "p1" not in _BUILD_CACHE:
        _BUILD_CACHE["p1"] = build_phase1()
    nc1 = _BUILD_CACHE["p1"] if _prebuilt is None else _prebuilt[0]
    in1 = []
    for c in range(NCORES):
        shard = xt[c * TPC:(c + 1) * TPC]
        in1.append({"xT": np.ascontiguousarray(shard.T), "wg": Wg})
    r1 = _run(nc1, in1, trace=_trace)
    ti = np.concatenate([r1.results[c]["ti"] for c in range(NCORES)], axis=0)
    tw = np.concatenate([r1.results[c]["tw"] for c in range(NCORES)], axis=0)
    global LAST_TI
    LAST_TI = ti
    stats = (np.stack([r1.results[c]["stats"] for c in range(NCORES)])
             .sum(axis=0).reshape(2, 128, E).sum(axis=1))

    # ---- host dispatch: group token slots by expert id ----
    eids = ti.astype(np.int64).ravel()
    wts = tw.ravel()
    toks = np.repeat(np.arange(T, dtype=np.int64), TOPK)
    perm = np.argsort(eids, kind="stable")
    s_tok = toks[perm]
    s_w = wts[perm]
    counts = np.bincount(eids, minlength=E)
    offs = np.zeros(E + 1, dtype=np.int64)
    np.cumsum(counts, out=offs[1:])

    # pair heavy experts with light ones so per-core work is balanced, and
    # size each slot's capacity to the max count it has to hold
    order = np.argsort(-counts, kind="stable")
    assign = [[int(order[c]), int(order[2 * NCORES - 1 - c])]
              for c in range(NCORES)]
    nmax = [int(max(counts[a[j]] for a in assign)) for j in range(EPC)]
    caps = [max(128, -(-n // 128) * 128) for n in nmax]

    # cores whose slot-j expert actually fills the last output m-tile; by
    # construction slot 0 counts descend with core id and slot 1 counts
    # ascend, so the run-set is an interval usable as a pid comparison
    mrun = []
    for j in range(EPC):
        need = [c for c in range(NCORES)
                if counts[assign[c][j]] > caps[j] - 128]
        iv = (min(need), max(need))
        mrun.append(iv if (len(need) == iv[1] - iv[0] + 1
                           and (iv[0] == 0 or iv[1] == NCORES - 1)) else None)

    use_b1 = bool(np.any(b1))
    use_b2 = bool(np.any(b2))
    key = (tuple(caps), tuple(nmax), tuple(mrun), use_b1, use_b2)
    if key not in _BUILD_CACHE:
        _BUILD_CACHE[key] = build_phase2(caps, use_b1, use_b2, nmax=nmax,
                                         mrun=mrun)
    nc2 = _BUILD_CACHE[key] if _prebuilt is None else _prebuilt[1]
    bf16 = ml_dtypes.bfloat16
    sel_of = {}
    in2 = []
    for c in range(NCORES):
        m = {"sg": stats.reshape(1, 2 * E)}
        for j in range(EPC):
            e = assign[c][j]
            sel = s_tok[offs[e]:offs[e + 1]]
            sel_of[e] = sel
            xg = np.zeros((H, caps[j]), dtype=bf16)
            xg[:, :len(sel)] = xt[sel].T.astype(bf16)
            gwv = np.zeros((1, caps[j]), dtype=np.float32)
            gwv[0, :len(sel)] = s_w[offs[e]:offs[e + 1]]
            m[f"xgT{j}"] = xg
            m[f"gw{j}"] = gwv
        es = [assign[c][0], assign[c][1]]
        m["w1"] = np.ascontiguousarray(W1[es]).astype(bf16)
        m["b1"] = np.ascontiguousarray(b1[es])
        m["w2"] = np.ascontiguousarray(W2[es]).astype(bf16)
        m["b2"] = np.ascontiguousarray(b2[es]).astype(bf16)
        in2.append(m)
    r2 = _run(nc2, in2, trace=_trace)

    # ---- host unshard: scatter-add weighted expert outputs ----
    out = np.zeros((T, H), dtype=np.float32)
    for c in range(NCORES):
        for j in range(EPC):
            e = assign[c][j]
            sel = sel_of[e]
            out[sel] += r2.results[c][f"yg{j}"][:len(sel)]
    aux = np.float32(r2.results[0]["aux"][0, 0])
    return out.reshape(B, S, H), aux


# revision 75
# speedup vs baseline: 1.0010x; 1.0010x over previous
"""MoE routing + expert MLP (DynSIHAMLP) on 8 TRN2 NeuronCores.

Strategy (expert-parallel, two device phases):
  Phase 1 (data-parallel router): each core takes T/8 = 2048 tokens,
    computes logits = x @ Wg in fp32 on the TensorEngine, softmax + top-2
    + renormalized weights on Vector/Scalar engines, and per-expert
    count/prob partial sums for the aux loss.
  Host dispatch: tokens are grouped by expert id (pure data movement),
    padded to a per-expert capacity chosen from the actual max count.
  Phase 2 (expert-parallel FFN): core c owns experts {2c, 2c+1}; computes
    y = silu(x @ W1 + b1) @ W2 + b2, scaled by the combine weight, in bf16
    on the TensorEngine (fp32 PSUM accumulation). Core 0 also computes the
    aux loss from the globally-summed router stats.
  Host unshard: scatter-add the two weighted expert outputs per token.
"""

import time

import numpy as np
import ml_dtypes
from contextlib import ExitStack

import concourse.bass as bass
import concourse.bacc as bacc
import concourse.mybir as mybir
import concourse.tile as tile
from concourse import bass_utils

dt = mybir.dt
AF = mybir.ActivationFunctionType
ALU = mybir.AluOpType
AX = mybir.AxisListType

B, S, H, E, F = 4, 4096, 1024, 16, 4096
T = B * S              # 16384 tokens
TOPK = 2
NCORES = 8
TPC = T // NCORES      # 2048 tokens per core in phase 1
EPC = E // NCORES      # 2 experts per core in phase 2
HK = H // 128          # 8 K-tiles over H
FK = F // 128          # 32 tiles over F
NT = TPC // 128        # 16 token tiles per core in phase 1


def build_phase1(xgrp=2, first_split=False, inc_out=True):
    """Router: per-core logits, softmax, top-2, weights, aux partial sums.

    xgrp: token tiles per x-load DMA (trade startup latency vs DMA count).
    first_split: load the first group's tiles individually so the first
      router matmul starts after a half-size DMA.
    """
    nc = bacc.Bacc("TRN2", target_bir_lowering=False, debug=False,
                   num_devices=NCORES)
    xT = nc.dram_tensor("xT", [H, TPC], dt.float32, kind="ExternalInput").ap()
    wg = nc.dram_tensor("wg", [H, E], dt.float32, kind="ExternalInput").ap()
    ti = nc.dram_tensor("ti", [TPC, TOPK], dt.uint32, kind="ExternalOutput").ap()
    tw = nc.dram_tensor("tw", [TPC, TOPK], dt.float32, kind="ExternalOutput").ap()
    # per-partition partial sums; the host finishes the reduction while
    # unsharding (rows 0..127 = dispatch counts, 128..255 = prob sums)
    stats = nc.dram_tensor("stats", [2 * 128, E], dt.float32,
                           kind="ExternalOutput").ap()

    with tile.TileContext(nc) as tc:
        with ExitStack() as ctx:
            const = ctx.enter_context(tc.tile_pool(name="const", bufs=1))
            work = ctx.enter_context(tc.tile_pool(name="work", bufs=6))
            accp = ctx.enter_context(tc.tile_pool(name="accp", bufs=1))
            psum = ctx.enter_context(tc.tile_pool(name="psum", bufs=4, space="PSUM"))
            psum_s = ctx.enter_context(tc.tile_pool(name="psum_s", bufs=1, space="PSUM"))

            wgs = const.tile([128, HK, E], dt.float32)
            nc.sync.dma_start(wgs[:], wg.rearrange("(hk p) e -> p hk e", p=128))
            xts = const.tile([128, NT // xgrp, HK, 128 * xgrp], dt.float32)
            for g in range(NT // xgrp):
                if g == 0 and first_split:
                    for s in range(xgrp):
                        nc.sync.dma_start(
                            xts[:, 0, :, s * 128:(s + 1) * 128],
                            xT[:, s * 128:(s + 1) * 128]
                            .rearrange("(hk p) t -> p hk t", p=128))
                    continue
                nc.sync.dma_start(
                    xts[:, g],
                    xT[:, g * 128 * xgrp:(g + 1) * 128 * xgrp]
                    .rearrange("(hk p) t -> p hk t", p=128))

            iota_i = const.tile([128, E], dt.int32)
            nc.gpsimd.iota(iota_i[:], pattern=[[1, E]], base=0, channel_multiplier=0)
            iota_f = const.tile([128, E], dt.float32)
            nc.vector.tensor_copy(iota_f[:], iota_i[:])

            cnt_acc = accp.tile([128, E], dt.float32)
            nc.vector.memset(cnt_acc[:], 0.0)
            prob_acc = accp.tile([128, E], dt.float32)
            nc.vector.memset(prob_acc[:], 0.0)
            ti_all = accp.tile([128, NT, TOPK], dt.uint32)
            tw_all = accp.tile([128, NT, TOPK], dt.float32)

            for i in range(NT):
                pl = psum.tile([128, E], dt.float32)
                for hk in range(HK):
                    nc.tensor.matmul(
                        pl[:],
                        xts[:, i // xgrp, hk,
                            (i % xgrp) * 128:(i % xgrp + 1) * 128],
                        wgs[:, hk, :],
                        start=(hk == 0), stop=(hk == HK - 1),
                    )
                negmax = work.tile([128, 1], dt.float32)
                nc.vector.tensor_reduce(negmax[:], pl[:], axis=AX.X,
                                        op=ALU.max, negate=True)
                expt = work.tile([128, E], dt.float32)
                nc.scalar.activation(expt[:], pl[:], AF.Exp,
                                     bias=negmax[:], scale=1.0)
                ssum = work.tile([128, 1], dt.float32)
                nc.vector.reduce_sum(ssum[:], expt[:], axis=AX.X)
                rs = work.tile([128, 1], dt.float32)
                nc.vector.reciprocal(rs[:], ssum[:])
                # prob_acc += expt * rs, fused; top-k runs in the exp domain
                # (same order as probs; the 1/Z factor cancels in tw)
                nc.vector.scalar_tensor_tensor(
                    prob_acc[:], expt[:], rs[:], prob_acc[:],
                    ALU.mult, ALU.add)

                vals8 = work.tile([128, 8], dt.float32)
                nc.vector.max(vals8[:], expt[:])
                idx8 = work.tile([128, 8], dt.uint32)
                nc.vector.max_index(idx8[:], vals8[:], expt[:])
                nc.vector.tensor_copy(ti_all[:, i, :], idx8[:, 0:TOPK])

                s2 = work.tile([128, 1], dt.float32)
                nc.vector.reduce_sum(s2[:], vals8[:, 0:TOPK], axis=AX.X)
                rs2 = work.tile([128, 1], dt.float32)
                nc.vector.reciprocal(rs2[:], s2[:])
                nc.scalar.mul(tw_all[:, i, :], vals8[:, 0:TOPK], rs2[:])

                idxf = work.tile([128, TOPK], dt.float32)
                nc.vector.tensor_copy(idxf[:], idx8[:, 0:TOPK])
                for k in range(TOPK):
                    nc.vector.scalar_tensor_tensor(
                        cnt_acc[:], iota_f[:], idxf[:, k:k + 1], cnt_acc[:],
                        ALU.is_equal, ALU.add)

                if inc_out and i % 4 == 3:
                    s = slice(i - 3, i + 1)
                    nc.sync.dma_start(
                        ti.rearrange("(i p) k -> p i k", p=128)[:, s], ti_all[:, s])
                    nc.sync.dma_start(
                        tw.rearrange("(i p) k -> p i k", p=128)[:, s], tw_all[:, s])

            nc.sync.dma_start(stats[0:128, :], cnt_acc[:])
            nc.sync.dma_start(stats[128:256, :], prob_acc[:])
            if not inc_out:
                nc.sync.dma_start(ti.rearrange("(i p) k -> p i k", p=128), ti_all[:])
                nc.sync.dma_start(tw.rearrange("(i p) k -> p i k", p=128), tw_all[:])
    nc.compile()
    return nc


def build_phase2(caps, use_b1=True, use_b2=True, bufs=(2, 3, 4, 4), repeat=1,
                 nq=32, nmax=None, mrun=None, smf=False):
    """Expert FFN: 2 experts per core, bf16 matmuls, fp32 accumulation.

    caps: per-slot token capacities (EPC ints, multiples of 128, compile-time).
    use_b1/use_b2: emit bias adds (skipped when the biases are all zero).
    nmax: per-slot exact max valid-token counts; mm1/activations stop there
      (only sound when every slot's first chunk is full, i.e. cap >= 512,
      so stale hs columns hold finite bf16 from a prior full chunk).
    mrun: per-slot optional (lo, hi) core-id interval that needs the slot's
      LAST output m-tile; other cores branch over it (their yg rows beyond
      their own token count stay zero and are never read by the host).
    """
    if isinstance(caps, int):
        caps = [caps] * EPC
    if nmax is None or any(n < 512 for n in nmax):
        nmax = list(caps)
    if mrun is None:
        mrun = [None] * EPC
    sf0 = smf
    nc = bacc.Bacc("TRN2", target_bir_lowering=False, debug=False,
                   num_devices=NCORES)
    xgT, gw, yg = [], [], []
    for j in range(EPC):
        xgT.append(nc.dram_tensor(f"xgT{j}", [H, caps[j]], dt.bfloat16,
                                  kind="ExternalInput").ap())
        gw.append(nc.dram_tensor(f"gw{j}", [1, caps[j]], dt.float32,
                                 kind="ExternalInput").ap())
        yg.append(nc.dram_tensor(f"yg{j}", [caps[j], H], dt.float32,
                                 kind="ExternalOutput").ap())
    w1 = nc.dram_tensor("w1", [EPC, H, F], dt.bfloat16, kind="ExternalInput").ap()
    b1 = nc.dram_tensor("b1", [EPC, F], dt.float32, kind="ExternalInput").ap()
    w2 = nc.dram_tensor("w2", [EPC, F, H], dt.bfloat16, kind="ExternalInput").ap()
    b2 = nc.dram_tensor("b2", [EPC, H], dt.bfloat16, kind="ExternalInput").ap()
    sg = nc.dram_tensor("sg", [1, 2 * E], dt.float32, kind="ExternalInput").ap()
    aux = nc.dram_tensor("aux", [1, 1], dt.float32, kind="ExternalOutput").ap()

    def chunk_list(cap, small_first=False):
        chunks, off = [], 0
        if small_first and cap > 512:
            chunks.append((0, 128))
            off = 128
        while off < cap:
            cn = min(512, cap - off)
            chunks.append((off, cn))
            off += cn
        return chunks

    with tile.TileContext(nc) as tc:
        with ExitStack() as ctx:
            xb, yb, p1b, p2b = bufs
            wpool = ctx.enter_context(tc.tile_pool(name="wpool", bufs=1))
            cpool = ctx.enter_context(tc.tile_pool(name="cpool", bufs=1))
            xpool = ctx.enter_context(tc.tile_pool(name="xpool", bufs=xb))
            hpool = ctx.enter_context(tc.tile_pool(name="hpool", bufs=1))
            ypool = ctx.enter_context(tc.tile_pool(name="ypool", bufs=yb))
            spool = ctx.enter_context(tc.tile_pool(name="spool", bufs=2))
            psum1 = ctx.enter_context(tc.tile_pool(name="psum1", bufs=p1b, space="PSUM"))
            psum2 = ctx.enter_context(tc.tile_pool(name="psum2", bufs=p2b, space="PSUM"))

            if any(iv is not None and (iv[0] > 0 or iv[1] < NCORES - 1)
                   for iv in mrun):
                nc.cache_partition_id()

            if use_b2:
                ones_bf = cpool.tile([1, 128], dt.bfloat16)
                nc.vector.memset(ones_bf[:], 1.0)

            # aux loss on every core (host reads core 0's)
            sgt = cpool.tile([1, 2 * E], dt.float32)
            nc.sync.dma_start(sgt[:], sg[:])
            prod = cpool.tile([1, E], dt.float32)
            nc.vector.tensor_tensor(prod[:], sgt[:, 0:E], sgt[:, E:2 * E], ALU.mult)
            psc = cpool.tile([1, 1], dt.float32)
            nc.vector.reduce_sum(psc[:], prod[:], axis=AX.X)
            auxt = cpool.tile([1, 1], dt.float32)
            nc.scalar.mul(auxt[:], psc[:], float(E) / (TOPK * T * T))
            nc.sync.dma_start(aux[:], auxt[:])

            NQ = nq           # W1 split into NQ pieces along F
            FQ = FK // NQ     # f-tiles per W1 piece
            FH = FK // 2      # W2 split into halves along F
            for e in [e for _ in range(repeat) for e in range(EPC)]:
                def w1_quarter(q, e=e):
                    t = wpool.tile([128, HK, F // NQ], dt.bfloat16, tag=f"w1q{q}")
                    nc.sync.dma_start(
                        t[:],
                        w1[e, :, q * (F // NQ):(q + 1) * (F // NQ)]
                        .rearrange("(hk p) f -> p hk f", p=128))
                    return t

                # only the first W1 quarter gates the first matmul; the rest
                # of the weights stream in behind the first token chunk
                w1q = [w1_quarter(0)]
                w2h = []
                first = True
                # hs per expert (later chunks' unwritten tail columns must
                # alias the previous chunk's finite values with tracked
                # dependencies), split in two f-halves so mm2's first half
                # doesn't wait on the last f-tile's activation
                hs_a = hpool.tile([128, FH, 512], dt.bfloat16, tag="hs_a")
                hs_b = hpool.tile([128, FH, 512], dt.bfloat16, tag="hs_b")
                hs2 = [hs_a, hs_b]
                for (c0, cn) in chunk_list(caps[e], small_first=(e == 0 and sf0)):
                    # valid-token count for this chunk (cv <= cn; pads beyond
                    # cv are skipped in mm1, zeroed by gw=0 in mm2's output)
                    cv = max(1, min(cn, nmax[e] - c0))
                    xt = xpool.tile([128, HK, 512], dt.bfloat16, tag="xt")
                    nc.sync.dma_start(
                        xt[:, :, 0:cv],
                        xgT[e][:, c0:c0 + cv].rearrange("(hk p) t -> p hk t", p=128))
                    gwt = spool.tile([128, 4], dt.float32, tag="gwt")
                    nm = cn // 128
                    nc.sync.dma_start(
                        gwt[:, 0:nm],
                        gw[e][0, c0:c0 + cn].rearrange("(s p) -> p s", p=128))

                    if first:
                        first = False
                        for q in range(1, NQ):
                            w1q.append(w1_quarter(q))
                        for half in range(2):
                            w2t = wpool.tile([128, FH, H], dt.bfloat16,
                                             tag=f"w2t{half}")
                            nc.sync.dma_start(
                                w2t[:],
                                w2[e, half * (F // 2):(half + 1) * (F // 2), :]
                                .rearrange("(fk p) h -> p fk h", p=128))
                            w2h.append(w2t)
                        if use_b1:
                            b1t = spool.tile([128, FK], dt.float32, tag="b1t")
                            nc.sync.dma_start(
                                b1t[:], b1[e].rearrange("(fk p) -> p fk", p=128))
                        if use_b2:
                            b2t = spool.tile([1, H], dt.bfloat16, tag="b2t")
                            nc.sync.dma_start(b2t[:], b2[e:e + 1, :])

                    for ft in range(FK):
                        w1s = w1q[ft // FQ][:, :, (ft % FQ) * 128:(ft % FQ + 1) * 128]
                        ph = psum1.tile([128, 512], dt.float32, tag="ph")
                        for hk in range(HK):
                            nc.tensor.matmul(
                                ph[:, 0:cv],
                                w1s[:, hk, :],
                                xt[:, hk, 0:cv],
                                start=(hk == 0), stop=(hk == HK - 1))
                        # silu(z) = z * sigmoid(z), z = ph + b1
                        b1ap = b1t[:, ft:ft + 1] if use_b1 else 0.0
                        sig = ypool.tile([128, 512], dt.float32, tag="sig")
                        nc.scalar.activation(sig[:, 0:cv], ph[:, 0:cv],
                                             AF.Sigmoid, bias=b1ap,
                                             scale=1.0)
                        nc.vector.scalar_tensor_tensor(
                            hs2[ft // FH][:, ft % FH, 0:cv], ph[:, 0:cv], b1ap,
                            sig[:, 0:cv], ALU.add, ALU.mult)

                    for m in range(nm):
                        def mtile(m=m, c0=c0, e=e, gwt=gwt, hs2=hs2, w2h=w2h,
                                  b2t=(b2t if use_b2 else None)):
                            for nh in range(2):
                                py = psum2.tile([128, 512], dt.float32, tag="py")
                                for fk in range(FK):
                                    nc.tensor.matmul(
                                        py[:],
                                        hs2[fk // FH][:, fk % FH,
                                                      m * 128:(m + 1) * 128],
                                        w2h[fk // FH][:, fk % FH,
                                                      nh * 512:(nh + 1) * 512],
                                        start=(fk == 0),
                                        stop=(not use_b2 and fk == FK - 1))
                                if use_b2:
                                    nc.tensor.matmul(
                                        py[:], ones_bf[:],
                                        b2t[:, nh * 512:(nh + 1) * 512],
                                        start=False, stop=True)
                                ysb = ypool.tile([128, 512], dt.float32, tag="ysb")
                                nc.scalar.mul(ysb[:], py[:], gwt[:, m:m + 1])
                                nc.sync.dma_start(
                                    yg[e][c0 + m * 128:c0 + (m + 1) * 128,
                                          nh * 512:(nh + 1) * 512],
                                    ysb[:])

                        is_last_mtile = (c0 + (m + 1) * 128 == caps[e])
                        iv = mrun[e]
                        if (is_last_mtile and iv is not None
                                and (iv[0] > 0 or iv[1] < NCORES - 1)):
                            pid = nc.partition_id()
                            cond = (pid < iv[1] + 1 if iv[0] == 0
                                    else pid > iv[0] - 1)
                            with tc.If(cond):
                                mtile()
                        else:
                            mtile()
    nc.compile()
    return nc


LAST_EXEC_NS = []
LAST_TRACES = []
LAST_TI = None
_BUILD_CACHE = {}


def _run(nc, in_maps, **kw):
    last_exc = None
    for attempt in range(3):
        if attempt:
            time.sleep(30 * attempt)
        try:
            r = bass_utils.run_bass_kernel_spmd(
                nc, in_maps, core_ids=list(range(NCORES)), **kw)
            break
        except Exception as exc:  # transient device wedges (NRT_* unrecoverable)
            last_exc = exc
    else:
        raise last_exc
    if r.exec_time_ns is not None:
        LAST_EXEC_NS.append(r.exec_time_ns)
    if r.instructions_and_trace is not None:
        LAST_TRACES.append(r.instructions_and_trace[1])
    return r


def kernel(x, Wg, W1, b1, W2, b2, _trace=False, _prebuilt=None):
    LAST_EXEC_NS.clear()
    LAST_TRACES.clear()
    x = np.asarray(x, dtype=np.float32)
    Wg = np.asarray(Wg, dtype=np.float32)
    W1 = np.asarray(W1, dtype=np.float32)
    b1 = np.asarray(b1, dtype=np.float32)
    W2 = np.asarray(W2, dtype=np.float32)
    b2 = np.asarray(b2, dtype=np.float32)
    xt = x.reshape(T, H)

    # ---- phase 1: router ----
    if "p1" not in _BUILD_CACHE:
        _BUILD_CACHE["p1"] = build_phase1()
    nc1 = _BUILD_CACHE["p1"] if _prebuilt is None else _prebuilt[0]
    in1 = []
    for c in range(NCORES):
        shard = xt[c * TPC:(c + 1) * TPC]
        in1.append({"xT": np.ascontiguousarray(shard.T), "wg": Wg})
    r1 = _run(nc1, in1, trace=_trace)
    ti = np.concatenate([r1.results[c]["ti"] for c in range(NCORES)], axis=0)
    tw = np.concatenate([r1.results[c]["tw"] for c in range(NCORES)], axis=0)
    global LAST_TI
    LAST_TI = ti
    stats = (np.stack([r1.results[c]["stats"] for c in range(NCORES)])
             .sum(axis=0).reshape(2, 128, E).sum(axis=1))

    # ---- host dispatch: group token slots by expert id ----
    eids = ti.astype(np.int64).ravel()
    wts = tw.ravel()
    toks = np.repeat(np.arange(T, dtype=np.int64), TOPK)
    perm = np.argsort(eids, kind="stable")
    s_tok = toks[perm]
    s_w = wts[perm]
    counts = np.bincount(eids, minlength=E)
    offs = np.zeros(E + 1, dtype=np.int64)
    np.cumsum(counts, out=offs[1:])

    # pair heavy experts with light ones so per-core work is balanced, and
    # size each slot's capacity to the max count it has to hold
    order = np.argsort(-counts, kind="stable")
    assign = [[int(order[c]), int(order[2 * NCORES - 1 - c])]
              for c in range(NCORES)]
    nmax = [int(max(counts[a[j]] for a in assign)) for j in range(EPC)]
    caps = [max(128, -(-n // 128) * 128) for n in nmax]

    # cores whose slot-j expert actually fills the last output m-tile; by
    # construction slot 0 counts descend with core id and slot 1 counts
    # ascend, so the run-set is an interval usable as a pid comparison
    mrun = []
    for j in range(EPC):
        need = [c for c in range(NCORES)
                if counts[assign[c][j]] > caps[j] - 128]
        iv = (min(need), max(need))
        mrun.append(iv if (len(need) == iv[1] - iv[0] + 1
                           and (iv[0] == 0 or iv[1] == NCORES - 1)) else None)

    use_b1 = bool(np.any(b1))
    use_b2 = bool(np.any(b2))
    key = (tuple(caps), tuple(nmax), tuple(mrun), use_b1, use_b2)
    if key not in _BUILD_CACHE:
        _BUILD_CACHE[key] = build_phase2(caps, use_b1, use_b2, nmax=nmax,
                                         mrun=mrun)
    nc2 = _BUILD_CACHE[key] if _prebuilt is None else _prebuilt[1]
    bf16 = ml_dtypes.bfloat16
    sel_of = {}
    in2 = []
    for c in range(NCORES):
        m = {"sg": stats.reshape(1, 2 * E)}
        for j in range(EPC):
            e = assign[c][j]
            sel = s_tok[offs[e]:offs[e + 1]]
            sel_of[e] = sel
            xg = np.zeros((H, caps[j]), dtype=bf16)
            xg[:, :len(sel)] = xt[sel].T.astype(bf16)
            gwv = np.zeros((1, caps[j]), dtype=np.float32)
            gwv[0, :len(sel)] = s_w[offs[e]:offs[e + 1]]
            m[f"xgT{j}"] = xg
            m[f"gw{j}"] = gwv
        es = [assign[c][0], assign[c][1]]
        m["w1"] = np.ascontiguousarray(W1[es]).astype(bf16)
        m["b1"] = np.ascontiguousarray(b1[es])
        m["w2"] = np.ascontiguousarray(W2[es]).astype(bf16)
        m["b2"] = np.ascontiguousarray(b2[es]).astype(bf16)
        in2.append(m)
    r2 = _run(nc2, in2, trace=_trace)

    # ---- host unshard: scatter-add weighted expert outputs ----
    out = np.zeros((T, H), dtype=np.float32)
    for c in range(NCORES):
        for j in range(EPC):
            e = assign[c][j]
            sel = sel_of[e]
            out[sel] += r2.results[c][f"yg{j}"][:len(sel)]
    aux = np.float32(r2.results[0]["aux"][0, 0])
    return out.reshape(B, S, H), aux


# revision 76
# speedup vs baseline: 1.0020x; 1.0010x over previous
"""MoE routing + expert MLP (DynSIHAMLP) on 8 TRN2 NeuronCores.

Strategy (expert-parallel, two device phases):
  Phase 1 (data-parallel router): each core takes T/8 = 2048 tokens,
    computes logits = x @ Wg in fp32 on the TensorEngine, softmax + top-2
    + renormalized weights on Vector/Scalar engines, and per-expert
    count/prob partial sums for the aux loss.
  Host dispatch: tokens are grouped by expert id (pure data movement),
    padded to a per-expert capacity chosen from the actual max count.
  Phase 2 (expert-parallel FFN): core c owns experts {2c, 2c+1}; computes
    y = silu(x @ W1 + b1) @ W2 + b2, scaled by the combine weight, in bf16
    on the TensorEngine (fp32 PSUM accumulation). Core 0 also computes the
    aux loss from the globally-summed router stats.
  Host unshard: scatter-add the two weighted expert outputs per token.
"""

import time

import numpy as np
import ml_dtypes
from contextlib import ExitStack

import concourse.bass as bass
import concourse.bacc as bacc
import concourse.mybir as mybir
import concourse.tile as tile
from concourse import bass_utils

dt = mybir.dt
AF = mybir.ActivationFunctionType
ALU = mybir.AluOpType
AX = mybir.AxisListType

B, S, H, E, F = 4, 4096, 1024, 16, 4096
T = B * S              # 16384 tokens
TOPK = 2
NCORES = 8
TPC = T // NCORES      # 2048 tokens per core in phase 1
EPC = E // NCORES      # 2 experts per core in phase 2
HK = H // 128          # 8 K-tiles over H
FK = F // 128          # 32 tiles over F
NT = TPC // 128        # 16 token tiles per core in phase 1


def build_phase1(xgrp=2, first_split=False, inc_out=True):
    """Router: per-core logits, softmax, top-2, weights, aux partial sums.

    xgrp: token tiles per x-load DMA (trade startup latency vs DMA count).
    first_split: load the first group's tiles individually so the first
      router matmul starts after a half-size DMA.
    """
    nc = bacc.Bacc("TRN2", target_bir_lowering=False, debug=False,
                   num_devices=NCORES)
    xT = nc.dram_tensor("xT", [H, TPC], dt.float32, kind="ExternalInput").ap()
    wg = nc.dram_tensor("wg", [H, E], dt.float32, kind="ExternalInput").ap()
    ti = nc.dram_tensor("ti", [TPC, TOPK], dt.uint32, kind="ExternalOutput").ap()
    tw = nc.dram_tensor("tw", [TPC, TOPK], dt.float32, kind="ExternalOutput").ap()
    # per-partition partial sums; the host finishes the reduction while
    # unsharding (rows 0..127 = dispatch counts, 128..255 = prob sums)
    stats = nc.dram_tensor("stats", [2 * 128, E], dt.float32,
                           kind="ExternalOutput").ap()

    with tile.TileContext(nc) as tc:
        with ExitStack() as ctx:
            const = ctx.enter_context(tc.tile_pool(name="const", bufs=1))
            work = ctx.enter_context(tc.tile_pool(name="work", bufs=6))
            accp = ctx.enter_context(tc.tile_pool(name="accp", bufs=1))
            psum = ctx.enter_context(tc.tile_pool(name="psum", bufs=4, space="PSUM"))
            psum_s = ctx.enter_context(tc.tile_pool(name="psum_s", bufs=1, space="PSUM"))

            wgs = const.tile([128, HK, E], dt.float32)
            nc.sync.dma_start(wgs[:], wg.rearrange("(hk p) e -> p hk e", p=128))
            xts = const.tile([128, NT // xgrp, HK, 128 * xgrp], dt.float32)
            for g in range(NT // xgrp):
                if g == 0 and first_split:
                    for s in range(xgrp):
                        nc.sync.dma_start(
                            xts[:, 0, :, s * 128:(s + 1) * 128],
                            xT[:, s * 128:(s + 1) * 128]
                            .rearrange("(hk p) t -> p hk t", p=128))
                    continue
                nc.sync.dma_start(
                    xts[:, g],
                    xT[:, g * 128 * xgrp:(g + 1) * 128 * xgrp]
                    .rearrange("(hk p) t -> p hk t", p=128))

            iota_i = const.tile([128, E], dt.int32)
            nc.gpsimd.iota(iota_i[:], pattern=[[1, E]], base=0, channel_multiplier=0)
            iota_f = const.tile([128, E], dt.float32)
            nc.vector.tensor_copy(iota_f[:], iota_i[:])

            cnt_acc = accp.tile([128, E], dt.float32)
            nc.vector.memset(cnt_acc[:], 0.0)
            prob_acc = accp.tile([128, E], dt.float32)
            nc.vector.memset(prob_acc[:], 0.0)
            ti_all = accp.tile([128, NT, TOPK], dt.uint32)
            tw_all = accp.tile([128, NT, TOPK], dt.float32)

            for i in range(NT):
                pl = psum.tile([128, E], dt.float32)
                for hk in range(HK):
                    nc.tensor.matmul(
                        pl[:],
                        xts[:, i // xgrp, hk,
                            (i % xgrp) * 128:(i % xgrp + 1) * 128],
                        wgs[:, hk, :],
                        start=(hk == 0), stop=(hk == HK - 1),
                    )
                negmax = work.tile([128, 1], dt.float32)
                nc.vector.tensor_reduce(negmax[:], pl[:], axis=AX.X,
                                        op=ALU.max, negate=True)
                expt = work.tile([128, E], dt.float32)
                nc.scalar.activation(expt[:], pl[:], AF.Exp,
                                     bias=negmax[:], scale=1.0)
                ssum = work.tile([128, 1], dt.float32)
                nc.vector.reduce_sum(ssum[:], expt[:], axis=AX.X)
                rs = work.tile([128, 1], dt.float32)
                nc.vector.reciprocal(rs[:], ssum[:])
                # prob_acc += expt * rs, fused; top-k runs in the exp domain
                # (same order as probs; the 1/Z factor cancels in tw)
                nc.vector.scalar_tensor_tensor(
                    prob_acc[:], expt[:], rs[:], prob_acc[:],
                    ALU.mult, ALU.add)

                vals8 = work.tile([128, 8], dt.float32)
                nc.vector.max(vals8[:], expt[:])
                idx8 = work.tile([128, 8], dt.uint32)
                nc.vector.max_index(idx8[:], vals8[:], expt[:])
                nc.vector.tensor_copy(ti_all[:, i, :], idx8[:, 0:TOPK])

                s2 = work.tile([128, 1], dt.float32)
                nc.vector.reduce_sum(s2[:], vals8[:, 0:TOPK], axis=AX.X)
                rs2 = work.tile([128, 1], dt.float32)
                nc.vector.reciprocal(rs2[:], s2[:])
                nc.scalar.mul(tw_all[:, i, :], vals8[:, 0:TOPK], rs2[:])

                idxf = work.tile([128, TOPK], dt.float32)
                nc.vector.tensor_copy(idxf[:], idx8[:, 0:TOPK])
                for k in range(TOPK):
                    nc.vector.scalar_tensor_tensor(
                        cnt_acc[:], iota_f[:], idxf[:, k:k + 1], cnt_acc[:],
                        ALU.is_equal, ALU.add)

                if inc_out and i % 4 == 3:
                    s = slice(i - 3, i + 1)
                    nc.sync.dma_start(
                        ti.rearrange("(i p) k -> p i k", p=128)[:, s], ti_all[:, s])
                    nc.sync.dma_start(
                        tw.rearrange("(i p) k -> p i k", p=128)[:, s], tw_all[:, s])

            nc.sync.dma_start(stats[0:128, :], cnt_acc[:])
            nc.sync.dma_start(stats[128:256, :], prob_acc[:])
            if not inc_out:
                nc.sync.dma_start(ti.rearrange("(i p) k -> p i k", p=128), ti_all[:])
                nc.sync.dma_start(tw.rearrange("(i p) k -> p i k", p=128), tw_all[:])
    nc.compile()
    return nc


def build_phase2(caps, use_b1=True, use_b2=True, bufs=(2, 3, 4, 4), repeat=1,
                 nq=32, nmax=None, mrun=None, smf=False):
    """Expert FFN: 2 experts per core, bf16 matmuls, fp32 accumulation.

    caps: per-slot token capacities (EPC ints, multiples of 128, compile-time).
    use_b1/use_b2: emit bias adds (skipped when the biases are all zero).
    nmax: per-slot exact max valid-token counts; mm1/activations stop there
      (only sound when every slot's first chunk is full, i.e. cap >= 512,
      so stale hs columns hold finite bf16 from a prior full chunk).
    mrun: per-slot optional (lo, hi) core-id interval that needs the slot's
      LAST output m-tile; other cores branch over it (their yg rows beyond
      their own token count stay zero and are never read by the host).
    """
    if isinstance(caps, int):
        caps = [caps] * EPC
    if nmax is None or any(n < 512 for n in nmax):
        nmax = list(caps)
    if mrun is None:
        mrun = [None] * EPC
    sf0 = smf
    nc = bacc.Bacc("TRN2", target_bir_lowering=False, debug=False,
                   num_devices=NCORES)
    xgT, gw, yg = [], [], []
    for j in range(EPC):
        xgT.append(nc.dram_tensor(f"xgT{j}", [H, caps[j]], dt.bfloat16,
                                  kind="ExternalInput").ap())
        gw.append(nc.dram_tensor(f"gw{j}", [1, caps[j]], dt.float32,
                                 kind="ExternalInput").ap())
        yg.append(nc.dram_tensor(f"yg{j}", [caps[j], H], dt.float32,
                                 kind="ExternalOutput").ap())
    w1 = nc.dram_tensor("w1", [EPC, H, F], dt.bfloat16, kind="ExternalInput").ap()
    b1 = nc.dram_tensor("b1", [EPC, F], dt.float32, kind="ExternalInput").ap()
    w2 = nc.dram_tensor("w2", [EPC, F, H], dt.bfloat16, kind="ExternalInput").ap()
    b2 = nc.dram_tensor("b2", [EPC, H], dt.bfloat16, kind="ExternalInput").ap()
    sg = nc.dram_tensor("sg", [1, 2 * E], dt.float32, kind="ExternalInput").ap()
    aux = nc.dram_tensor("aux", [1, 1], dt.float32, kind="ExternalOutput").ap()

    def chunk_list(cap, small_first=False):
        chunks, off = [], 0
        if small_first and cap > 512:
            chunks.append((0, 128))
            off = 128
        while off < cap:
            cn = min(512, cap - off)
            chunks.append((off, cn))
            off += cn
        return chunks

    with tile.TileContext(nc) as tc:
        with ExitStack() as ctx:
            xb, yb, p1b, p2b = bufs
            wpool = ctx.enter_context(tc.tile_pool(name="wpool", bufs=1))
            cpool = ctx.enter_context(tc.tile_pool(name="cpool", bufs=1))
            xpool = ctx.enter_context(tc.tile_pool(name="xpool", bufs=xb))
            hpool = ctx.enter_context(tc.tile_pool(name="hpool", bufs=1))
            ypool = ctx.enter_context(tc.tile_pool(name="ypool", bufs=yb))
            spool = ctx.enter_context(tc.tile_pool(name="spool", bufs=2))
            psum1 = ctx.enter_context(tc.tile_pool(name="psum1", bufs=p1b, space="PSUM"))
            psum2 = ctx.enter_context(tc.tile_pool(name="psum2", bufs=p2b, space="PSUM"))

            if any(iv is not None and (iv[0] > 0 or iv[1] < NCORES - 1)
                   for iv in mrun):
                nc.cache_partition_id()

            if use_b2:
                ones_bf = cpool.tile([1, 128], dt.bfloat16)
                nc.vector.memset(ones_bf[:], 1.0)

            # aux loss on every core (host reads core 0's)
            sgt = cpool.tile([1, 2 * E], dt.float32)
            nc.sync.dma_start(sgt[:], sg[:])
            prod = cpool.tile([1, E], dt.float32)
            nc.vector.tensor_tensor(prod[:], sgt[:, 0:E], sgt[:, E:2 * E], ALU.mult)
            psc = cpool.tile([1, 1], dt.float32)
            nc.vector.reduce_sum(psc[:], prod[:], axis=AX.X)
            auxt = cpool.tile([1, 1], dt.float32)
            nc.scalar.mul(auxt[:], psc[:], float(E) / (TOPK * T * T))
            nc.sync.dma_start(aux[:], auxt[:])

            NQ = nq           # W1 split into NQ pieces along F
            FQ = FK // NQ     # f-tiles per W1 piece
            FH = FK // 4      # W2 split into quarters along F
            for e in [e for _ in range(repeat) for e in range(EPC)]:
                def w1_quarter(q, e=e):
                    t = wpool.tile([128, HK, F // NQ], dt.bfloat16, tag=f"w1q{q}")
                    nc.sync.dma_start(
                        t[:],
                        w1[e, :, q * (F // NQ):(q + 1) * (F // NQ)]
                        .rearrange("(hk p) f -> p hk f", p=128))
                    return t

                # only the first W1 quarter gates the first matmul; the rest
                # of the weights stream in behind the first token chunk
                w1q = [w1_quarter(0)]
                w2h = []
                first = True
                # hs per expert (later chunks' unwritten tail columns must
                # alias the previous chunk's finite values with tracked
                # dependencies), split in two f-halves so mm2's first half
                # doesn't wait on the last f-tile's activation
                hs_a = hpool.tile([128, FK // 2, 512], dt.bfloat16, tag="hs_a")
                hs_b = hpool.tile([128, FK // 2, 512], dt.bfloat16, tag="hs_b")
                hs2 = [hs_a, hs_b]
                for (c0, cn) in chunk_list(caps[e], small_first=(e == 0 and sf0)):
                    # valid-token count for this chunk (cv <= cn; pads beyond
                    # cv are skipped in mm1, zeroed by gw=0 in mm2's output)
                    cv = max(1, min(cn, nmax[e] - c0))
                    xt = xpool.tile([128, HK, 512], dt.bfloat16, tag="xt")
                    nc.sync.dma_start(
                        xt[:, :, 0:cv],
                        xgT[e][:, c0:c0 + cv].rearrange("(hk p) t -> p hk t", p=128))
                    gwt = spool.tile([128, 4], dt.float32, tag="gwt")
                    nm = cn // 128
                    nc.sync.dma_start(
                        gwt[:, 0:nm],
                        gw[e][0, c0:c0 + cn].rearrange("(s p) -> p s", p=128))

                    if first:
                        first = False
                        for q in range(1, NQ):
                            w1q.append(w1_quarter(q))
                        for half in range(4):
                            w2t = wpool.tile([128, FH, H], dt.bfloat16,
                                             tag=f"w2t{half}")
                            nc.sync.dma_start(
                                w2t[:],
                                w2[e, half * (F // 4):(half + 1) * (F // 4), :]
                                .rearrange("(fk p) h -> p fk h", p=128))
                            w2h.append(w2t)
                        if use_b1:
                            b1t = spool.tile([128, FK], dt.float32, tag="b1t")
                            nc.sync.dma_start(
                                b1t[:], b1[e].rearrange("(fk p) -> p fk", p=128))
                        if use_b2:
                            b2t = spool.tile([1, H], dt.bfloat16, tag="b2t")
                            nc.sync.dma_start(b2t[:], b2[e:e + 1, :])

                    for ft in range(FK):
                        w1s = w1q[ft // FQ][:, :, (ft % FQ) * 128:(ft % FQ + 1) * 128]
                        ph = psum1.tile([128, 512], dt.float32, tag="ph")
                        for hk in range(HK):
                            nc.tensor.matmul(
                                ph[:, 0:cv],
                                w1s[:, hk, :],
                                xt[:, hk, 0:cv],
                                start=(hk == 0), stop=(hk == HK - 1))
                        # silu(z) = z * sigmoid(z), z = ph + b1
                        b1ap = b1t[:, ft:ft + 1] if use_b1 else 0.0
                        sig = ypool.tile([128, 512], dt.float32, tag="sig")
                        nc.scalar.activation(sig[:, 0:cv], ph[:, 0:cv],
                                             AF.Sigmoid, bias=b1ap,
                                             scale=1.0)
                        nc.vector.scalar_tensor_tensor(
                            hs2[ft // (FK // 2)][:, ft % (FK // 2), 0:cv], ph[:, 0:cv], b1ap,
                            sig[:, 0:cv], ALU.add, ALU.mult)

                    for m in range(nm):
                        def mtile(m=m, c0=c0, e=e, gwt=gwt, hs2=hs2, w2h=w2h,
                                  b2t=(b2t if use_b2 else None)):
                            for nh in range(2):
                                py = psum2.tile([128, 512], dt.float32, tag="py")
                                for fk in range(FK):
                                    nc.tensor.matmul(
                                        py[:],
                                        hs2[fk // (FK // 2)][:, fk % (FK // 2),
                                                      m * 128:(m + 1) * 128],
                                        w2h[fk // FH][:, fk % FH,
                                                      nh * 512:(nh + 1) * 512],
                                        start=(fk == 0),
                                        stop=(not use_b2 and fk == FK - 1))
                                if use_b2:
                                    nc.tensor.matmul(
                                        py[:], ones_bf[:],
                                        b2t[:, nh * 512:(nh + 1) * 512],
                                        start=False, stop=True)
                                ysb = ypool.tile([128, 512], dt.float32, tag="ysb")
                                nc.scalar.mul(ysb[:], py[:], gwt[:, m:m + 1])
                                nc.sync.dma_start(
                                    yg[e][c0 + m * 128:c0 + (m + 1) * 128,
                                          nh * 512:(nh + 1) * 512],
                                    ysb[:])

                        is_last_mtile = (c0 + (m + 1) * 128 == caps[e])
                        iv = mrun[e]
                        if (is_last_mtile and iv is not None
                                and (iv[0] > 0 or iv[1] < NCORES - 1)):
                            pid = nc.partition_id()
                            cond = (pid < iv[1] + 1 if iv[0] == 0
                                    else pid > iv[0] - 1)
                            with tc.If(cond):
                                mtile()
                        else:
                            mtile()
    nc.compile()
    return nc


LAST_EXEC_NS = []
LAST_TRACES = []
LAST_TI = None
_BUILD_CACHE = {}


def _run(nc, in_maps, **kw):
    last_exc = None
    for attempt in range(3):
        if attempt:
            time.sleep(30 * attempt)
        try:
            r = bass_utils.run_bass_kernel_spmd(
                nc, in_maps, core_ids=list(range(NCORES)), **kw)
            break
        except Exception as exc:  # transient device wedges (NRT_* unrecoverable)
            last_exc = exc
    else:
        raise last_exc
    if r.exec_time_ns is not None:
        LAST_EXEC_NS.append(r.exec_time_ns)
    if r.instructions_and_trace is not None:
        LAST_TRACES.append(r.instructions_and_trace[1])
    return r


def kernel(x, Wg, W1, b1, W2, b2, _trace=False, _prebuilt=None):
    LAST_EXEC_NS.clear()
    LAST_TRACES.clear()
    x = np.asarray(x, dtype=np.float32)
    Wg = np.asarray(Wg, dtype=np.float32)
    W1 = np.asarray(W1, dtype=np.float32)
    b1 = np.asarray(b1, dtype=np.float32)
    W2 = np.asarray(W2, dtype=np.float32)
    b2 = np.asarray(b2, dtype=np.float32)
    xt = x.reshape(T, H)

    # ---- phase 1: router ----
    if "p1" not in _BUILD_CACHE:
        _BUILD_CACHE["p1"] = build_phase1()
    nc1 = _BUILD_CACHE["p1"] if _prebuilt is None else _prebuilt[0]
    in1 = []
    for c in range(NCORES):
        shard = xt[c * TPC:(c + 1) * TPC]
        in1.append({"xT": np.ascontiguousarray(shard.T), "wg": Wg})
    r1 = _run(nc1, in1, trace=_trace)
    ti = np.concatenate([r1.results[c]["ti"] for c in range(NCORES)], axis=0)
    tw = np.concatenate([r1.results[c]["tw"] for c in range(NCORES)], axis=0)
    global LAST_TI
    LAST_TI = ti
    stats = (np.stack([r1.results[c]["stats"] for c in range(NCORES)])
             .sum(axis=0).reshape(2, 128, E).sum(axis=1))

    # ---- host dispatch: group token slots by expert id ----
    eids = ti.astype(np.int64).ravel()
    wts = tw.ravel()
    toks = np.repeat(np.arange(T, dtype=np.int64), TOPK)
    perm = np.argsort(eids, kind="stable")
    s_tok = toks[perm]
    s_w = wts[perm]
    counts = np.bincount(eids, minlength=E)
    offs = np.zeros(E + 1, dtype=np.int64)
    np.cumsum(counts, out=offs[1:])

    # pair heavy experts with light ones so per-core work is balanced, and
    # size each slot's capacity to the max count it has to hold
    order = np.argsort(-counts, kind="stable")
    assign = [[int(order[c]), int(order[2 * NCORES - 1 - c])]
              for c in range(NCORES)]
    nmax = [int(max(counts[a[j]] for a in assign)) for j in range(EPC)]
    caps = [max(128, -(-n // 128) * 128) for n in nmax]

    # cores whose slot-j expert actually fills the last output m-tile; by
    # construction slot 0 counts descend with core id and slot 1 counts
    # ascend, so the run-set is an interval usable as a pid comparison
    mrun = []
    for j in range(EPC):
        need = [c for c in range(NCORES)
                if counts[assign[c][j]] > caps[j] - 128]
        iv = (min(need), max(need))
        mrun.append(iv if (len(need) == iv[1] - iv[0] + 1
                           and (iv[0] == 0 or iv[1] == NCORES - 1)) else None)

    use_b1 = bool(np.any(b1))
    use_b2 = bool(np.any(b2))
    key = (tuple(caps), tuple(nmax), tuple(mrun), use_b1, use_b2)
    if key not in _BUILD_CACHE:
        _BUILD_CACHE[key] = build_phase2(caps, use_b1, use_b2, nmax=nmax,
                                         mrun=mrun)
    nc2 = _BUILD_CACHE[key] if _prebuilt is None else _prebuilt[1]
    bf16 = ml_dtypes.bfloat16
    sel_of = {}
    in2 = []
    for c in range(NCORES):
        m = {"sg": stats.reshape(1, 2 * E)}
        for j in range(EPC):
            e = assign[c][j]
            sel = s_tok[offs[e]:offs[e + 1]]
            sel_of[e] = sel
            xg = np.zeros((H, caps[j]), dtype=bf16)
            xg[:, :len(sel)] = xt[sel].T.astype(bf16)
            gwv = np.zeros((1, caps[j]), dtype=np.float32)
            gwv[0, :len(sel)] = s_w[offs[e]:offs[e + 1]]
            m[f"xgT{j}"] = xg
            m[f"gw{j}"] = gwv
        es = [assign[c][0], assign[c][1]]
        m["w1"] = np.ascontiguousarray(W1[es]).astype(bf16)
        m["b1"] = np.ascontiguousarray(b1[es])
        m["w2"] = np.ascontiguousarray(W2[es]).astype(bf16)
        m["b2"] = np.ascontiguousarray(b2[es]).astype(bf16)
        in2.append(m)
    r2 = _run(nc2, in2, trace=_trace)

    # ---- host unshard: scatter-add weighted expert outputs ----
    out = np.zeros((T, H), dtype=np.float32)
    for c in range(NCORES):
        for j in range(EPC):
            e = assign[c][j]
            sel = sel_of[e]
            out[sel] += r2.results[c][f"yg{j}"][:len(sel)]
    aux = np.float32(r2.results[0]["aux"][0, 0])
    return out.reshape(B, S, H), aux
